# revision 19
# baseline (speedup 1.0000x reference)
"""BinaryConvBNReLU Trainium2 kernel (8 NeuronCores, data-parallel over batch).

Reference computation (per nn.Module):
  bx = sign(x);  wc = clip(w, -1, 1);  alpha = mean(|wc|);  bw = sign(wc) * alpha
  out = conv2d(bx, bw, stride 1, pad 1) + x          (identity shortcut)
  out = batchnorm(out, batch stats over (B, H, W), gamma, beta, eps=1e-5)
  y = relu(out)

Strategy:
  - Batch sharded 4 images/core; weights replicated per core.
  - conv(sign x, sign w) on TensorE as 9 shifted fp8 DoubleRow matmuls per
    output-channel chunk (+-1 exact in fp8e4; contract dim 256 per matmul via
    [128, 2, N] paired operands; PSUM accumulates exact integers). The padded
    sign(x) image is stored flat (58-wide rows) so each tap's moving operand
    is one contiguous run; row-wrap garbage lands only in the 2 padding
    columns of each output row, skipped at PSUM eviction.
  - alpha (mean |clip(w)|) folded in at eviction: out = alpha*psum + x, with
    the per-channel BN sum fused in (accum_out); sum-of-squares on ScalarE.
  - Sync-BN via AllGather (4.6us floor vs AllReduce's 9.7us): each core
    contributes [128,2] (sum, sumsq) per chunk; the gathered [8*128,2] is
    read back as 8 slot DMAs fanned across the sync/scalar/gpsimd rings and
    rank-reduced on DVE. Stats for chunk 0 collect at 50% of the conv so its
    normalize+store fully hides inside the chunk-1 conv; chunk 1's AllGather
    is the only exposed collective (its stream is warm by then).
  - Normalized output is written to a separate bf16 buffer and stored as
    bf16 (half the store bytes; host upcasts to fp32 - rel tol is 2e-2).
    Scratch (sumsq dump, weight-clip) aliases into dead regions of the bf16
    buffer to stay within SBUF.
  - Startup: both weight DMAs lead the fast sync ring (w1 feeds alpha which
    gates the first eviction; on the slow gpsimd ring it stalled PSUM ->
    TensorE at t=18-27us), then image 0 in quarter slices so sign(x) and the
    tap-pipelined w-sign -> transpose -> fp8-copy chain put the first matmul
    at ~8.5us. Store DMAs issue from the engine that produced each slice so
    the sync ring never queues a stats DMA behind them.
"""

import numpy as np

B, C, H, W = 32, 256, 56, 56
K = 3
EPS = 1e-5
N_CORES = 8
B_LOC = B // N_CORES          # 4 images per core
P = 128                       # SBUF partitions
NCH = C // P                  # 2 channel chunks
HW = H * W                    # 3136
HP, WP = H + 2, W + 2         # 58x58 zero-padded sign(x) layout
ROWS = 8                      # output rows per PSUM tile
NRT = H // ROWS               # 7 row tiles per image
NT = ROWS * W                 # 448 pixels per PSUM tile
COUNT = B * HW                # BN reduction count (global batch)

_CACHE = {}


def _build_nc():
    import concourse.bacc as bacc
    import concourse.bass_isa as bass_isa
    import concourse.mybir as mybir
    import concourse.tile as tile
    from concourse.masks import make_identity
    from contextlib import ExitStack

    f32 = mybir.dt.float32
    bf16 = mybir.dt.bfloat16
    f8 = mybir.dt.float8e4
    Alu = mybir.AluOpType
    Act = mybir.ActivationFunctionType
    AxisX = mybir.AxisListType.X
    DR = mybir.MatmulPerfMode.DoubleRow

    # flat padded sign(x) layout: BASE leading zeros + 58*58 image (+ tail pad)
    # so every (kh, kw) tap window is one contiguous run (row-wrap garbage only
    # pollutes the 2 padding columns, which eviction skips). XLEN % 16 == 0
    # keeps the fp8 DoubleRow pair-step constraint satisfied.
    BASE = 16
    XLEN = 3392  # 16 + 58*58 + 12

    nc = bacc.Bacc(
        "TRN2", target_bir_lowering=False, debug=False, num_devices=N_CORES
    )
    x_d = nc.dram_tensor("x", [B_LOC, C, H, W], f32, kind="ExternalInput")
    w_d = nc.dram_tensor("w", [C, C, K, K], f32, kind="ExternalInput")
    g_d = nc.dram_tensor("gamma", [C], f32, kind="ExternalInput")
    be_d = nc.dram_tensor("beta", [C], f32, kind="ExternalInput")
    y_d = nc.dram_tensor("y", [B_LOC, C, H, W], bf16, kind="ExternalOutput")

    with tile.TileContext(nc) as tc, ExitStack() as es:
        big = es.enter_context(tc.tile_pool(name="big", bufs=1))
        wpool = es.enter_context(tc.tile_pool(name="wpool", bufs=1))
        wst = es.enter_context(tc.tile_pool(name="wst", bufs=2))
        sgt = es.enter_context(tc.tile_pool(name="sgt", bufs=2))
        xpadp = es.enter_context(tc.tile_pool(name="xpadp", bufs=B_LOC))
        psum = es.enter_context(tc.tile_pool(name="psum", bufs=6, space="PSUM"))
        psum_t = es.enter_context(tc.tile_pool(name="psum_t", bufs=2, space="PSUM"))
        dram = es.enter_context(tc.tile_pool(name="dram", bufs=1, space="DRAM"))

        # x (then conv+x) stays resident in SBUF fp32; the normalized ReLU
        # output goes to a separate bf16 buffer (half-size stores).
        out_sb = big.tile([P, B_LOC, NCH, HW], f32, name="out_sb")
        y_sb = big.tile([P, B_LOC, NCH, HW], bf16, name="y_sb")
        # Transposed sign weights for fp8 DoubleRow: [ci_local, tap, ci_chunk, co].
        wT8 = wpool.tile([P, K * K, NCH, C], f8, name="wT8")
        identity = wpool.tile([P, P], bf16, name="identity")
        make_identity(nc, identity)

        gamma_sb = wpool.tile([P, NCH], f32, name="gamma_sb")
        beta_sb = wpool.tile([P, NCH], f32, name="beta_sb")

        sum_stat = wpool.tile([P, NCH, B_LOC * NRT], f32, name="sum_stat")
        # per chunk: entries 0..5 = half-image squares (imgs 0-2, 2 halves
        # each; accum_out is per-instruction so each ACT needs its own slot);
        # 6..6+NRT-1 = per-row-tile squares of img 3 (AllGather critical path).
        sq_stat = wpool.tile([P, NCH, 6 + NRT], f32, name="sq_stat")
        eps_sb = wpool.tile([P, 1], f32, name="eps_sb")
        nc.vector.memset(eps_sb[:], EPS)
        neg_eps = wpool.tile([P, 1], f32, name="neg_eps")
        nc.vector.memset(neg_eps[:], -EPS)

        w_flat = w_d.ap().rearrange("o i kh kw -> o (i kh kw)")
        a_parts = wpool.tile([P, NCH], f32, name="a_parts")
        x_flat = x_d.ap().rearrange("b c h w -> b c (h w)")
        y_flat = y_d.ap().rearrange("b c h w -> b c (h w)")
        stats_loc = wpool.tile([P, NCH, 2], f32, name="stats_loc")
        pre_stat = wpool.tile([P, NCH, 2], f32, name="pre_stat")

        w_sbs = [
            wst.tile([P, C * K * K], f32, tag="wsb", name=f"wsb{j}")
            for j in range(NCH)
        ]
        # scratch aliases into y_sb regions whose apply-writes happen strictly
        # after the scratch's last use (chunk-1 applies start only after the
        # final AllGather):
        #   sq dump target <- y_sb img0/chunk1; clip(|w|) <- y_sb img1/chunk1+
        y_lin = y_sb.rearrange("p b j n -> p (b j n)")
        sq_scr = y_sb[:, 0, 1, :].bitcast(f32)  # [P, HW//2] f32
        wclip = y_lin[:, 3 * HW : 3 * HW + 2 * C * K * K].bitcast(f32)

        def w_dma(j):
            # w0 leads the sync ring (it gates the whole sign->transpose->
            # cast->matmul startup chain; giving it dedicated ring bandwidth
            # for its first 3.3us beats racing the x loads). w1 rides the
            # scalar ring concurrently - it only gates alpha, needed by the
            # first PSUM eviction at ~20us.
            eng = nc.sync if j == 0 else nc.scalar
            eng.dma_start(w_sbs[j][:], w_flat[j * P : (j + 1) * P, :])

        def w_sign_tap(j, sgn, t):
            w_taps = w_sbs[j].rearrange("p (c t) -> p t c", t=K * K)
            nc.scalar.activation(sgn[:, t, :], w_taps[:, t, :], Act.Sign)

        def w_transpose_tap(j, sgn, t):
            pts = []
            for k in range(NCH):
                pt = psum_t.tile([P, P], bf16, tag="pt", name=f"pt{j}_{t}_{k}")
                nc.tensor.transpose(pt[:], sgn[:, t, k * P : (k + 1) * P], identity[:])
                pts.append((t, k, pt))
            return pts

        def w_transpose_copy(j, pts):
            # PSUM->SBUF copy casts to fp8; on DVE so ScalarE stays
            # free for the x sign passes
            for t, k, pt in pts:
                nc.vector.tensor_copy(wT8[:, t, k, j * P : (j + 1) * P], pt[:])

        def w_clip_reduce(j):
            nc.vector.tensor_scalar(
                wclip, w_sbs[j][:], 1.0, -1.0, Alu.min, Alu.max
            )
            nc.vector.tensor_reduce(
                a_parts[:, j : j + 1],
                wclip,
                axis=AxisX,
                op=Alu.add,
                apply_absolute_value=True,
            )

        xpads = [
            xpadp.tile([P, NCH, XLEN], f8, tag="xpad", name=f"xpad{b}")
            for b in range(B_LOC)
        ]

        def halo_memset(b):
            # zero only the cells the conv taps actually read as padding:
            # BASE prefix + top padded row; the 2-wide column seam between
            # consecutive rows ((r,57),(r+1,0) are contiguous); bottom padded
            # row + tail. ~270 cells/partition vs 6784 for a full memset.
            xp = xpads[b]
            nc.gpsimd.memset(xp[:, :, 0 : BASE + WP], 0.0)
            seam = xp[:, :, BASE + WP - 1 : BASE + WP - 1 + 56 * WP].rearrange(
                "p k (r c) -> p k r c", c=WP
            )[:, :, :, 0:2]
            nc.gpsimd.memset(seam, 0.0)
            nc.gpsimd.memset(xp[:, :, BASE + 56 * WP + WP - 1 : XLEN], 0.0)

        def x_load_rows(b, r0, r1):
            for k in range(NCH):
                nc.sync.dma_start(
                    out_sb[:, b, k, r0 * W : r1 * W],
                    x_flat[b, k * P : (k + 1) * P, r0 * W : r1 * W],
                )

        def x_sign_rows(b, r0, r1):
            xpad = xpads[b]
            for k in range(NCH):
                pad_img = xpad[:, k, BASE : BASE + HP * WP].rearrange(
                    "p (r c) -> p r c", c=WP
                )
                nc.scalar.activation(
                    pad_img[:, r0 + 1 : r1 + 1, 1 : W + 1],
                    out_sb[:, b, k, r0 * W : r1 * W].rearrange(
                        "p (h w) -> p h w", w=W
                    ),
                    Act.Sign,
                )

        # alpha on DVE+gpsimd only (ACT stays free for sign passes)
        a_sum = wpool.tile([P, 1], f32, name="a_sum")
        a_all = wpool.tile([P, 1], f32, name="a_all")
        alpha = wpool.tile([P, 1], f32, name="alpha")

        def alpha_finalize():
            nc.vector.tensor_reduce(a_sum[:], a_parts[:], axis=AxisX, op=Alu.add)
            nc.gpsimd.partition_all_reduce(
                a_all[:], a_sum[:], channels=P, reduce_op=bass_isa.ReduceOp.add
            )
            nc.vector.tensor_scalar_mul(alpha[:], a_all[:], 1.0 / (C * C * K * K))

        def sq_big(j, b, slot):
            # per-half-image sum-of-squares ACTs (ScalarE); dump to scratch.
            # accum_out is per-instruction, so each half gets its own slot.
            HH_ = HW // 2
            nc.scalar.activation(
                sq_scr, out_sb[:, b, j, 0:HH_], Act.Square,
                accum_out=sq_stat[:, j, 2 * slot : 2 * slot + 1],
            )
            nc.scalar.activation(
                sq_scr, out_sb[:, b, j, HH_:HW], Act.Square,
                accum_out=sq_stat[:, j, 2 * slot + 1 : 2 * slot + 2],
            )

        def sq_tiles(j, b):
            # per-row-tile squares: keeps the LAST unit's sumsq off a 2.7us
            # whole-image ACT on the AllGather critical path
            for rt in range(NRT):
                nc.scalar.activation(
                    sq_scr[:, 0:NT],
                    out_sb[:, b, j, rt * NT : (rt + 1) * NT],
                    Act.Square,
                    accum_out=sq_stat[:, j, 6 + rt : 7 + rt],
                )

        def conv_matmuls(j, b, mid_cb=None):
            xpad = xpads[b]
            tiles = []
            for rt in range(NRT):
                if rt == 4 and mid_cb is not None:
                    mid_cb()
                # padded-width output tile [8 rows, 58 cols]; cols 0 and 57 are
                # row-wrap garbage and are skipped at eviction.
                ps = psum.tile([P, ROWS, WP], f32, tag="ps", name=f"ps{b}_{j}_{rt}")
                mm = 0
                for kh in range(K):
                    for kw in range(K):
                        s = BASE + (rt * ROWS + kh) * WP + (kw - 1)
                        nc.tensor.matmul(
                            ps[:],
                            wT8[:, kh * K + kw, :, j * P : (j + 1) * P],
                            xpad[:, :, s : s + ROWS * WP],
                            start=(mm == 0),
                            stop=(mm == K * K - 1),
                            perf_mode=DR,
                        )
                        mm += 1
                tiles.append(ps)
            return tiles

        def conv_evict(j, b, tiles, mid_cb=None):
            for rt in range(NRT):
                if rt == 3 and mid_cb is not None:
                    mid_cb()
                idx = b * NRT + rt
                sl = out_sb[:, b, j, rt * NT : (rt + 1) * NT].rearrange(
                    "p (r c) -> p r c", c=W
                )
                # out = alpha*conv + x (in place over x), accum -> per-tile sum
                nc.vector.scalar_tensor_tensor(
                    out=sl,
                    in0=tiles[rt][:, :, 1 : W + 1],
                    scalar=alpha[:],
                    in1=sl,
                    op0=Alu.mult,
                    op1=Alu.add,
                    accum_out=sum_stat[:, j, idx : idx + 1],
                )

        def stat_prereduce(j):
            # fold images 0-2 (21 sum entries + 3 sq entries) into partials
            # during the conv so the post-conv reduce covers only image 3
            nc.vector.tensor_reduce(
                pre_stat[:, j, 0:1], sum_stat[:, j, 0:21], axis=AxisX, op=Alu.add
            )
            nc.vector.tensor_reduce(
                pre_stat[:, j, 1:2], sq_stat[:, j, 0:6], axis=AxisX, op=Alu.add
            )

        def stat_reduce(j):
            nc.vector.tensor_reduce(
                stats_loc[:, j, 0:1], sum_stat[:, j, 21:28], axis=AxisX, op=Alu.add
            )
            nc.vector.tensor_reduce(
                stats_loc[:, j, 1:2], sq_stat[:, j, 6:13], axis=AxisX, op=Alu.add
            )
            nc.vector.tensor_tensor(
                stats_loc[:, j, :], stats_loc[:, j, :], pre_stat[:, j, :], Alu.add
            )

        def launch_cc(j):
            bnc_in = dram.tile([P, 2], f32, name=f"bncin{j}")
            bnc_out = dram.tile(
                [N_CORES * P, 2], f32, name=f"bncout{j}", addr_space="Shared"
            )
            # sync ring: HWDGE completion is ~2us vs ~10us via SWDGE
            nc.sync.dma_start(bnc_in[:], stats_loc[:, j, :])
            nc.gpsimd.collective_compute(
                "AllGather",
                Alu.bypass,
                replica_groups=[list(range(N_CORES))],
                ins=[bnc_in.opt()],
                outs=[bnc_out.opt()],
            )
            return bnc_out

        # gathered per-rank stats [P, rank, 2]; slot DMAs fan across the sync
        # and gpsimd rings (both idle by collective-completion time; keeping
        # them off the scalar ring protects the conv-critical ACT FIFO)
        globs = wpool.tile([P, NCH, N_CORES, 2], f32, name="globs")

        def glob_ret(j, bnc_out):
            for r in range(N_CORES):
                eng = (nc.sync, nc.gpsimd)[r % 2]
                eng.dma_start(
                    globs[:, j, r, :], bnc_out[r * P : (r + 1) * P, :]
                )

        def norm_pre(j):
            # rank-reduce the gathered stats, then the DVE front half of the
            # BN chain; reciprocal runs BEFORE the single ACT hop (sqrt) so
            # the post-sqrt tail is short: rsqrt(v+eps) = sqrt(1/(v+eps))
            glob = wpool.tile([P, 2], f32, name=f"glob{j}")
            nc.vector.tensor_reduce(
                glob[:, 0:1], globs[:, j, :, 0], axis=AxisX, op=Alu.add
            )
            nc.vector.tensor_reduce(
                glob[:, 1:2], globs[:, j, :, 1], axis=AxisX, op=Alu.add
            )
            me = wpool.tile([P, 2], f32, name=f"me{j}")  # [mean, ex2]
            nc.vector.tensor_scalar_mul(me[:], glob[:], 1.0 / COUNT)
            mean = me[:, 0:1]
            # tmp = mean^2 - eps;  vpe = ex2 - tmp = var + eps;  rvpe = 1/vpe
            tmp = wpool.tile([P, 1], f32, name=f"tmp{j}")
            nc.vector.tensor_scalar(tmp[:], mean, mean, neg_eps[:], Alu.mult, Alu.add)
            vpe = wpool.tile([P, 1], f32, name=f"vpe{j}")
            nc.vector.tensor_sub(vpe[:], me[:, 1:2], tmp[:])
            rvpe = wpool.tile([P, 1], f32, name=f"rvpe{j}")
            nc.vector.reciprocal(rvpe[:], vpe[:])
            return mean, rvpe

        def norm_mid(j, rvpe):
            rsd = wpool.tile([P, 1], f32, name=f"rsd{j}")
            nc.scalar.activation(rsd[:], rvpe[:], Act.Sqrt)
            return rsd

        def norm_post(j, mean, rsd):
            scl = wpool.tile([P, 1], f32, name=f"scl{j}")
            nc.vector.tensor_mul(scl[:], rsd[:], gamma_sb[:, j : j + 1])
            mscl = wpool.tile([P, 1], f32, name=f"mscl{j}")
            nc.vector.tensor_mul(mscl[:], mean, scl[:])
            bia = wpool.tile([P, 1], f32, name=f"bia{j}")
            nc.vector.tensor_sub(bia[:], beta_sb[:, j : j + 1], mscl[:])
            return scl, bia

        HH = HW // 2

        def apply_slice(j, b, h, eng, scl, bia):
            # y = relu(scale*out + bias) -> bf16 y_sb (stores are emitted
            # separately on the sync/gpsimd rings). gpsimd elementwise is
            # ~10x slower than ACT/DVE and starves DVE's SBUF port - never
            # put apply work there.
            src = out_sb[:, b, j, h * HH : (h + 1) * HH]
            dst = y_sb[:, b, j, h * HH : (h + 1) * HH]
            if eng == "s":
                nc.scalar.activation(dst, src, Act.Relu, bias=bia[:], scale=scl[:])
            else:
                nc.vector.tensor_scalar(dst, src, scl[:], bia[:], Alu.mult, Alu.add)
                nc.vector.tensor_scalar_max(dst, dst, 0.0)

        def apply_chunk(j, scl, bia):
            # 8 half-image slices fanned ACT(4, fused relu) / DVE(4)
            plan = ["s", "v", "s", "v", "s", "v", "s", "v"]
            for i, eng in enumerate(plan):
                apply_slice(j, i // 2, i % 2, eng, scl, bia)

        def store_chunk(j, engs):
            # 8 half-image bf16 stores round-robined over otherwise-idle rings
            for i in range(8):
                b, h = i // 2, i % 2
                engs[i % len(engs)].dma_start(
                    y_flat[b, j * P : (j + 1) * P, h * HH : (h + 1) * HH],
                    y_sb[:, b, j, h * HH : (h + 1) * HH],
                )

        # ------------------------------------------------------------------
        # Emission. Every engine queue is strict in-order; the interleaving
        # below is the schedule (see module docstring for the timeline).
        # ------------------------------------------------------------------

        # gpsimd first: halo memsets (x_sign can't start until its xpad's
        # pad cells are zeroed; SWDGE gamma/beta issues must queue AFTER)
        for b in range(B_LOC):
            halo_memset(b)
        nc.gpsimd.dma_start(gamma_sb[:], g_d.ap().rearrange("(j p) -> p j", p=P))
        nc.gpsimd.dma_start(beta_sb[:], be_d.ap().rearrange("(j p) -> p j", p=P))
        # dummy warmup collective: absorbs the ~11us ncfw wakeup so the
        # real AllGathers start within ~1us of their triggers
        warm_in = dram.tile([P, 1], f32, name="warm_in")
        warm_out = dram.tile(
            [N_CORES * P, 1], f32, name="warm_out", addr_space="Shared"
        )
        nc.gpsimd.dma_start(warm_in[:], eps_sb[:])
        nc.gpsimd.collective_compute(
            "AllGather",
            Alu.bypass,
            replica_groups=[list(range(N_CORES))],
            ins=[warm_in.opt()],
            outs=[warm_out.opt()],
        )

        # sync ring: w0, w1 first (w1 -> alpha by ~13us; evictions need it at
        # ~20us), then img0 in quarter slices, imgs 1-3 in halves.
        w_dma(0)
        w_dma(1)
        Q = H // 4
        for q in range(4):
            x_load_rows(0, q * Q, (q + 1) * Q)
        for b in range(1, B_LOC):
            x_load_rows(b, 0, H // 2)
            x_load_rows(b, H // 2, H)

        # ScalarE: chunk-0 w signs back-to-back (transposes+fp8 casts
        # pipeline behind them on TE/DVE), then img0 signs at quarter grain.
        sgn0 = sgt.tile([P, K * K, C], bf16, tag="sgn", name="sgn0")
        pts0 = []
        for t in range(K * K):
            w_sign_tap(0, sgn0, t)
            pts0 += w_transpose_tap(0, sgn0, t)
        for q in range(4):
            x_sign_rows(0, q * Q, (q + 1) * Q)

        # DVE: the fp8 casts gate the first matmuls - they go FIRST; the clip
        # chains only feed alpha, needed at the first eviction (~10us later).
        # (clip TS ops wait on the w DMAs, which finish late because w and x
        # loads share HBM bandwidth - queueing them before the casts stalled
        # TensorE for 10.7us.)
        w_transpose_copy(0, pts0)
        w_clip_reduce(0)
        w_clip_reduce(1)
        alpha_finalize()
        # preload the Sqrt ACT table while ScalarE has slack so the BN norm
        # chain doesn't eat a 1.3us ACT_TABLE_LOAD on the post-collective
        # critical path
        sqrt_warm = wpool.tile([P, 1], f32, name="sqrt_warm")
        nc.scalar.activation(sqrt_warm[:], eps_sb[:], Act.Sqrt)

        # Unit order: all chunk-0 units first so AllGather(0) launches
        # mid-kernel; ALL chunk-1 conv/stats work is emitted before any
        # AllGather(0)-gated op so a late collective can never stall the
        # conv-critical ACT/DVE queues. Chunk-0 normalize+apply runs inside
        # the AllGather(1) wait; only its stores + chunk-1's tail are exposed.
        mm00 = conv_matmuls(0, 0)
        x_sign_rows(1, 0, H // 2)
        x_sign_rows(1, H // 2, H)
        conv_evict(0, 0, mm00)
        mm01 = conv_matmuls(0, 1)
        x_sign_rows(2, 0, H // 2)
        x_sign_rows(2, H // 2, H)
        conv_evict(0, 1, mm01)
        mm02 = conv_matmuls(0, 2)
        sq_big(0, 0, 0)
        x_sign_rows(3, 0, H // 2)
        x_sign_rows(3, H // 2, H)
        conv_evict(0, 2, mm02)
        sq_big(0, 1, 1)
        sgn1 = sgt.tile([P, K * K, C], bf16, tag="sgn", name="sgn1")
        for t in range(K * K):
            w_sign_tap(1, sgn1, t)
        mm03 = conv_matmuls(0, 3)
        conv_evict(0, 3, mm03)
        sq_big(0, 2, 2)
        stat_prereduce(0)
        sq_tiles(0, 3)
        stat_reduce(0)
        cc0 = launch_cc(0)

        pts1 = []
        for t in range(K * K):
            pts1 += w_transpose_tap(1, sgn1, t)
        w_transpose_copy(1, pts1)
        mm10 = conv_matmuls(1, 0)
        conv_evict(1, 0, mm10)
        sq_big(1, 0, 0)
        glob_ret(0, cc0)  # slot DMAs on sync+gpsimd, idle by now
        mm11 = conv_matmuls(1, 1)
        conv_evict(1, 1, mm11)
        sq_big(1, 1, 1)
        mm12 = conv_matmuls(1, 2)
        conv_evict(1, 2, mm12)
        sq_big(1, 2, 2)
        stat_prereduce(1)
        mm13 = conv_matmuls(1, 3)
        conv_evict(1, 3, mm13)
        sq_tiles(1, 3)
        stat_reduce(1)
        cc1 = launch_cc(1)

        # chunk-0 normalize+apply: emitted after the AllGather(1) launch so
        # every conv-critical op is already queued ahead of it; it executes
        # inside the AllGather(1) wait window.
        mean0, var0 = norm_pre(0)
        sd0 = norm_mid(0, var0)
        scl0, bia0 = norm_post(0, mean0, sd0)
        apply_chunk(0, scl0, bia0)
        store_chunk(0, [nc.sync])

        glob_ret(1, cc1)
        mean1, var1 = norm_pre(1)
        sd1 = norm_mid(1, var1)
        scl1, bia1 = norm_post(1, mean1, sd1)
        apply_chunk(1, scl1, bia1)
        # tail stores on sync+gpsimd so apply1's ACT slices never interleave
        # with store-issue instructions on the scalar queue
        store_chunk(1, [nc.sync, nc.gpsimd])

    nc.compile()
    return nc


def _get_nc():
    if "nc" not in _CACHE:
        _CACHE["nc"] = _build_nc()
    return _CACHE["nc"]


def _run(in_maps, trace=False, tmpdir=None):
    import concourse.bass_utils as bass_utils

    nc = _get_nc()
    return bass_utils.run_bass_kernel_spmd(
        nc, in_maps, core_ids=list(range(N_CORES)), trace=trace, tmpdir=tmpdir
    )


def _make_in_maps(x, w, gamma, beta):
    x = np.ascontiguousarray(np.asarray(x), dtype=np.float32)
    w = np.ascontiguousarray(np.asarray(w), dtype=np.float32)
    gamma = np.ascontiguousarray(np.asarray(gamma), dtype=np.float32)
    beta = np.ascontiguousarray(np.asarray(beta), dtype=np.float32)
    assert x.shape == (B, C, H, W)
    xs = np.split(x, N_CORES, axis=0)
    return [
        {"x": xs[i], "w": w, "gamma": gamma, "beta": beta} for i in range(N_CORES)
    ]


def _gather_y(res):
    ys = [np.asarray(r["y"]) for r in res.results]
    return np.concatenate(ys, axis=0).astype(np.float32)


def kernel(x, w, gamma, beta):
    in_maps = _make_in_maps(x, w, gamma, beta)
    res = _run(in_maps, trace=False)
    return _gather_y(res)


# ---- profiling helpers (used by test.py only) -------------------------

def _install_ntff_hook_shim():
    """bass_utils wants antenv.axon_hooks for NTFF tracing under axon; shim it."""
    import sys
    import types

    import antenv

    if "antenv.axon_hooks" in sys.modules:
        return
    mod = types.ModuleType("antenv.axon_hooks")
    mod._hook = None
    mod.set_axon_ntff_profile_hook = lambda h: setattr(mod, "_hook", h)
    mod.get_axon_ntff_profile_hook = lambda: mod._hook
    sys.modules["antenv.axon_hooks"] = mod
    antenv.axon_hooks = mod

    from trn_agent_boot.trn_boot import _ntff_profile_via_ctypes

    mod.set_axon_ntff_profile_hook(
        _ntff_profile_via_ctypes("/opt/axon/libaxon_pjrt.so")
    )


def kernel_traced(x, w, gamma, beta, tmpdir=None):
    """Run once with NTFF profiling; returns (y_full, exec_time_ns, trace_path)."""
    import concourse.bass_utils as bass_utils

    _install_ntff_hook_shim()
    bass_utils.upload_artifacts = lambda d: "local://disabled"
    in_maps = _make_in_maps(x, w, gamma, beta)
    res = _run(in_maps, trace=True, tmpdir=tmpdir)
    y = _gather_y(res)
    trace_path = (
        res.instructions_and_trace[1] if res.instructions_and_trace else None
    )
    return y, res.exec_time_ns, trace_path


# revision 20
# speedup vs baseline: 1.1492x; 1.1492x over previous
"""BinaryConvBNReLU Trainium2 kernel (8 NeuronCores, data-parallel over batch).

Reference computation (per nn.Module):
  bx = sign(x);  wc = clip(w, -1, 1);  alpha = mean(|wc|);  bw = sign(wc) * alpha
  out = conv2d(bx, bw, stride 1, pad 1) + x          (identity shortcut)
  out = batchnorm(out, batch stats over (B, H, W), gamma, beta, eps=1e-5)
  y = relu(out)

Strategy:
  - Batch sharded 4 images/core; weights replicated per core.
  - conv(sign x, sign w) on TensorE as 9 shifted fp8 DoubleRow matmuls per
    output-channel chunk (+-1 exact in fp8e4; contract dim 256 per matmul via
    [128, 2, N] paired operands; PSUM accumulates exact integers). The padded
    sign(x) image is stored flat (58-wide rows) so each tap's moving operand
    is one contiguous run; row-wrap garbage lands only in the 2 padding
    columns of each output row, skipped at PSUM eviction.
  - alpha (mean |clip(w)|) folded in at eviction: out = alpha*psum + x, with
    the per-channel BN sum fused in (accum_out); sum-of-squares on ScalarE.
  - Sync-BN via AllGather (4.6us floor vs AllReduce's 9.7us): each core
    contributes [128,2] (sum, sumsq) per chunk; the gathered [8*128,2] is
    read back as 8 slot DMAs fanned across the sync/scalar/gpsimd rings and
    rank-reduced on DVE. Stats for chunk 0 collect at 50% of the conv so its
    normalize+store fully hides inside the chunk-1 conv; chunk 1's AllGather
    is the only exposed collective (its stream is warm by then).
  - Normalized output is written to a separate bf16 buffer and stored as
    bf16 (half the store bytes; host upcasts to fp32 - rel tol is 2e-2).
    Scratch (sumsq dump, weight-clip) aliases into dead regions of the bf16
    buffer to stay within SBUF.
  - Startup: both weight DMAs lead the fast sync ring (w1 feeds alpha which
    gates the first eviction; on the slow gpsimd ring it stalled PSUM ->
    TensorE at t=18-27us), then image 0 in quarter slices so sign(x) and the
    tap-pipelined w-sign -> transpose -> fp8-copy chain put the first matmul
    at ~8.5us. Store DMAs issue from the engine that produced each slice so
    the sync ring never queues a stats DMA behind them.
"""

import numpy as np

B, C, H, W = 32, 256, 56, 56
K = 3
EPS = 1e-5
N_CORES = 8
B_LOC = B // N_CORES          # 4 images per core
P = 128                       # SBUF partitions
NCH = C // P                  # 2 channel chunks
HW = H * W                    # 3136
HP, WP = H + 2, W + 2         # 58x58 zero-padded sign(x) layout
ROWS = 8                      # output rows per PSUM tile
NRT = H // ROWS               # 7 row tiles per image
NT = ROWS * W                 # 448 pixels per PSUM tile
COUNT = B * HW                # BN reduction count (global batch)

_CACHE = {}


def _build_nc():
    import concourse.bacc as bacc
    import concourse.bass_isa as bass_isa
    import concourse.mybir as mybir
    import concourse.tile as tile
    from concourse.masks import make_identity
    from contextlib import ExitStack

    f32 = mybir.dt.float32
    bf16 = mybir.dt.bfloat16
    f8 = mybir.dt.float8e4
    Alu = mybir.AluOpType
    Act = mybir.ActivationFunctionType
    AxisX = mybir.AxisListType.X
    DR = mybir.MatmulPerfMode.DoubleRow

    # flat padded sign(x) layout: BASE leading zeros + 58*58 image (+ tail pad)
    # so every (kh, kw) tap window is one contiguous run (row-wrap garbage only
    # pollutes the 2 padding columns, which eviction skips). XLEN % 16 == 0
    # keeps the fp8 DoubleRow pair-step constraint satisfied.
    BASE = 16
    XLEN = 3392  # 16 + 58*58 + 12

    nc = bacc.Bacc(
        "TRN2", target_bir_lowering=False, debug=False, num_devices=N_CORES
    )
    x_d = nc.dram_tensor("x", [B_LOC, C, H, W], f32, kind="ExternalInput")
    w_d = nc.dram_tensor("w", [C, C, K, K], f32, kind="ExternalInput")
    g_d = nc.dram_tensor("gamma", [C], f32, kind="ExternalInput")
    be_d = nc.dram_tensor("beta", [C], f32, kind="ExternalInput")
    y_d = nc.dram_tensor("y", [B_LOC, C, H, W], bf16, kind="ExternalOutput")

    with tile.TileContext(nc) as tc, ExitStack() as es:
        big = es.enter_context(tc.tile_pool(name="big", bufs=1))
        wpool = es.enter_context(tc.tile_pool(name="wpool", bufs=1))
        wst = es.enter_context(tc.tile_pool(name="wst", bufs=2))
        sgt = es.enter_context(tc.tile_pool(name="sgt", bufs=2))
        xpadp = es.enter_context(tc.tile_pool(name="xpadp", bufs=B_LOC))
        psum = es.enter_context(tc.tile_pool(name="psum", bufs=6, space="PSUM"))
        psum_t = es.enter_context(tc.tile_pool(name="psum_t", bufs=2, space="PSUM"))
        dram = es.enter_context(tc.tile_pool(name="dram", bufs=1, space="DRAM"))

        # x (then conv+x) stays resident in SBUF fp32; the normalized ReLU
        # output goes to a separate bf16 buffer (half-size stores).
        out_sb = big.tile([P, B_LOC, NCH, HW], f32, name="out_sb")
        y_sb = big.tile([P, B_LOC, NCH, HW], bf16, name="y_sb")
        # Transposed sign weights for fp8 DoubleRow: [ci_local, tap, ci_chunk, co].
        wT8 = wpool.tile([P, K * K, NCH, C], f8, name="wT8")
        identity = wpool.tile([P, P], bf16, name="identity")
        make_identity(nc, identity)

        gamma_sb = wpool.tile([P, NCH], f32, name="gamma_sb")
        beta_sb = wpool.tile([P, NCH], f32, name="beta_sb")

        sum_stat = wpool.tile([P, NCH, B_LOC * NRT], f32, name="sum_stat")
        # per chunk: entries 0..5 = half-image squares (imgs 0-2, 2 halves
        # each; accum_out is per-instruction so each ACT needs its own slot);
        # 6..6+NRT-1 = per-row-tile squares of img 3 (AllGather critical path).
        sq_stat = wpool.tile([P, NCH, 6 + NRT], f32, name="sq_stat")
        eps_sb = wpool.tile([P, 1], f32, name="eps_sb")
        nc.vector.memset(eps_sb[:], EPS)
        neg_eps = wpool.tile([P, 1], f32, name="neg_eps")
        nc.vector.memset(neg_eps[:], -EPS)

        w_flat = w_d.ap().rearrange("o i kh kw -> o (i kh kw)")
        a_parts = wpool.tile([P, NCH], f32, name="a_parts")
        x_flat = x_d.ap().rearrange("b c h w -> b c (h w)")
        y_flat = y_d.ap().rearrange("b c h w -> b c (h w)")
        stats_loc = wpool.tile([P, NCH, 2], f32, name="stats_loc")
        pre_stat = wpool.tile([P, NCH, 2], f32, name="pre_stat")

        w_sbs = [
            wst.tile([P, C * K * K], f32, tag="wsb", name=f"wsb{j}")
            for j in range(NCH)
        ]
        # scratch aliases into y_sb regions whose apply-writes happen strictly
        # after the scratch's last use (chunk-1 applies start only after the
        # final AllGather):
        #   sq dump target <- y_sb img0/chunk1; clip(|w|) <- y_sb img1/chunk1+
        y_lin = y_sb.rearrange("p b j n -> p (b j n)")
        sq_scr = y_sb[:, 0, 1, :].bitcast(f32)  # [P, HW//2] f32
        wclip = y_lin[:, 3 * HW : 3 * HW + 2 * C * K * K].bitcast(f32)

        def w_dma(j):
            # both weight chunks ride the scalar ring (HWDGE, ACT issues them
            # as its first two ops) so the sync ring belongs entirely to the
            # x loads: w0 gates the startup chain, w1 gates alpha (needed by
            # the first eviction at ~20us), and x1's halves must land by
            # ~25us - putting w0 ahead of the x loads on one ring stalls the
            # conv at the img0/img1 boundary instead.
            nc.scalar.dma_start(w_sbs[j][:], w_flat[j * P : (j + 1) * P, :])

        def w_sign_tap(j, sgn, t):
            w_taps = w_sbs[j].rearrange("p (c t) -> p t c", t=K * K)
            nc.scalar.activation(sgn[:, t, :], w_taps[:, t, :], Act.Sign)

        def w_transpose_tap(j, sgn, t):
            pts = []
            for k in range(NCH):
                pt = psum_t.tile([P, P], bf16, tag="pt", name=f"pt{j}_{t}_{k}")
                nc.tensor.transpose(pt[:], sgn[:, t, k * P : (k + 1) * P], identity[:])
                pts.append((t, k, pt))
            return pts

        def w_transpose_copy(j, pts):
            # PSUM->SBUF copy casts to fp8; on DVE so ScalarE stays
            # free for the x sign passes
            for t, k, pt in pts:
                nc.vector.tensor_copy(wT8[:, t, k, j * P : (j + 1) * P], pt[:])

        def w_clip_reduce(j):
            nc.vector.tensor_scalar(
                wclip, w_sbs[j][:], 1.0, -1.0, Alu.min, Alu.max
            )
            nc.vector.tensor_reduce(
                a_parts[:, j : j + 1],
                wclip,
                axis=AxisX,
                op=Alu.add,
                apply_absolute_value=True,
            )

        xpads = [
            xpadp.tile([P, NCH, XLEN], f8, tag="xpad", name=f"xpad{b}")
            for b in range(B_LOC)
        ]

        def halo_memset(b):
            # zero only the cells the conv taps actually read as padding:
            # BASE prefix + top padded row; the 2-wide column seam between
            # consecutive rows ((r,57),(r+1,0) are contiguous); bottom padded
            # row + tail. ~270 cells/partition vs 6784 for a full memset.
            xp = xpads[b]
            nc.gpsimd.memset(xp[:, :, 0 : BASE + WP], 0.0)
            seam = xp[:, :, BASE + WP - 1 : BASE + WP - 1 + 56 * WP].rearrange(
                "p k (r c) -> p k r c", c=WP
            )[:, :, :, 0:2]
            nc.gpsimd.memset(seam, 0.0)
            nc.gpsimd.memset(xp[:, :, BASE + 56 * WP + WP - 1 : XLEN], 0.0)

        def x_load_rows(b, r0, r1):
            for k in range(NCH):
                nc.sync.dma_start(
                    out_sb[:, b, k, r0 * W : r1 * W],
                    x_flat[b, k * P : (k + 1) * P, r0 * W : r1 * W],
                )

        def x_sign_rows(b, r0, r1):
            xpad = xpads[b]
            for k in range(NCH):
                pad_img = xpad[:, k, BASE : BASE + HP * WP].rearrange(
                    "p (r c) -> p r c", c=WP
                )
                nc.scalar.activation(
                    pad_img[:, r0 + 1 : r1 + 1, 1 : W + 1],
                    out_sb[:, b, k, r0 * W : r1 * W].rearrange(
                        "p (h w) -> p h w", w=W
                    ),
                    Act.Sign,
                )

        # alpha on DVE+gpsimd only (ACT stays free for sign passes)
        a_sum = wpool.tile([P, 1], f32, name="a_sum")
        a_all = wpool.tile([P, 1], f32, name="a_all")
        alpha = wpool.tile([P, 1], f32, name="alpha")

        def alpha_finalize():
            nc.vector.tensor_reduce(a_sum[:], a_parts[:], axis=AxisX, op=Alu.add)
            nc.gpsimd.partition_all_reduce(
                a_all[:], a_sum[:], channels=P, reduce_op=bass_isa.ReduceOp.add
            )
            nc.vector.tensor_scalar_mul(alpha[:], a_all[:], 1.0 / (C * C * K * K))

        def sq_big(j, b, slot):
            # per-half-image sum-of-squares ACTs (ScalarE); dump to scratch.
            # accum_out is per-instruction, so each half gets its own slot.
            HH_ = HW // 2
            nc.scalar.activation(
                sq_scr, out_sb[:, b, j, 0:HH_], Act.Square,
                accum_out=sq_stat[:, j, 2 * slot : 2 * slot + 1],
            )
            nc.scalar.activation(
                sq_scr, out_sb[:, b, j, HH_:HW], Act.Square,
                accum_out=sq_stat[:, j, 2 * slot + 1 : 2 * slot + 2],
            )

        def sq_tiles(j, b):
            # per-row-tile squares: keeps the LAST unit's sumsq off a 2.7us
            # whole-image ACT on the AllGather critical path
            for rt in range(NRT):
                nc.scalar.activation(
                    sq_scr[:, 0:NT],
                    out_sb[:, b, j, rt * NT : (rt + 1) * NT],
                    Act.Square,
                    accum_out=sq_stat[:, j, 6 + rt : 7 + rt],
                )

        def conv_matmuls(j, b, mid_cb=None):
            xpad = xpads[b]
            tiles = []
            for rt in range(NRT):
                if rt == 4 and mid_cb is not None:
                    mid_cb()
                # padded-width output tile [8 rows, 58 cols]; cols 0 and 57 are
                # row-wrap garbage and are skipped at eviction.
                ps = psum.tile([P, ROWS, WP], f32, tag="ps", name=f"ps{b}_{j}_{rt}")
                mm = 0
                for kh in range(K):
                    for kw in range(K):
                        s = BASE + (rt * ROWS + kh) * WP + (kw - 1)
                        nc.tensor.matmul(
                            ps[:],
                            wT8[:, kh * K + kw, :, j * P : (j + 1) * P],
                            xpad[:, :, s : s + ROWS * WP],
                            start=(mm == 0),
                            stop=(mm == K * K - 1),
                            perf_mode=DR,
                        )
                        mm += 1
                tiles.append(ps)
            return tiles

        def conv_evict(j, b, tiles, mid_cb=None):
            for rt in range(NRT):
                if rt == 3 and mid_cb is not None:
                    mid_cb()
                idx = b * NRT + rt
                sl = out_sb[:, b, j, rt * NT : (rt + 1) * NT].rearrange(
                    "p (r c) -> p r c", c=W
                )
                # out = alpha*conv + x (in place over x), accum -> per-tile sum
                nc.vector.scalar_tensor_tensor(
                    out=sl,
                    in0=tiles[rt][:, :, 1 : W + 1],
                    scalar=alpha[:],
                    in1=sl,
                    op0=Alu.mult,
                    op1=Alu.add,
                    accum_out=sum_stat[:, j, idx : idx + 1],
                )

        def stat_prereduce(j):
            # fold images 0-2 (21 sum entries + 3 sq entries) into partials
            # during the conv so the post-conv reduce covers only image 3
            nc.vector.tensor_reduce(
                pre_stat[:, j, 0:1], sum_stat[:, j, 0:21], axis=AxisX, op=Alu.add
            )
            nc.vector.tensor_reduce(
                pre_stat[:, j, 1:2], sq_stat[:, j, 0:6], axis=AxisX, op=Alu.add
            )

        def stat_reduce(j):
            nc.vector.tensor_reduce(
                stats_loc[:, j, 0:1], sum_stat[:, j, 21:28], axis=AxisX, op=Alu.add
            )
            nc.vector.tensor_reduce(
                stats_loc[:, j, 1:2], sq_stat[:, j, 6:13], axis=AxisX, op=Alu.add
            )
            nc.vector.tensor_tensor(
                stats_loc[:, j, :], stats_loc[:, j, :], pre_stat[:, j, :], Alu.add
            )

        def launch_cc(j):
            bnc_in = dram.tile([P, 2], f32, name=f"bncin{j}")
            bnc_out = dram.tile(
                [N_CORES * P, 2], f32, name=f"bncout{j}", addr_space="Shared"
            )
            # sync ring: HWDGE completion is ~2us vs ~10us via SWDGE
            nc.sync.dma_start(bnc_in[:], stats_loc[:, j, :])
            nc.gpsimd.collective_compute(
                "AllGather",
                Alu.bypass,
                replica_groups=[list(range(N_CORES))],
                ins=[bnc_in.opt()],
                outs=[bnc_out.opt()],
            )
            return bnc_out

        # gathered per-rank stats [P, rank, 2]; slot DMAs fan across the sync
        # and gpsimd rings (both idle by collective-completion time; keeping
        # them off the scalar ring protects the conv-critical ACT FIFO)
        globs = wpool.tile([P, NCH, N_CORES, 2], f32, name="globs")

        def glob_ret(j, bnc_out):
            for r in range(N_CORES):
                eng = (nc.sync, nc.gpsimd)[r % 2]
                eng.dma_start(
                    globs[:, j, r, :], bnc_out[r * P : (r + 1) * P, :]
                )

        def norm_pre(j):
            # rank-reduce the gathered stats, then the DVE front half of the
            # BN chain; reciprocal runs BEFORE the single ACT hop (sqrt) so
            # the post-sqrt tail is short: rsqrt(v+eps) = sqrt(1/(v+eps))
            glob = wpool.tile([P, 2], f32, name=f"glob{j}")
            nc.vector.tensor_reduce(
                glob[:, 0:1], globs[:, j, :, 0], axis=AxisX, op=Alu.add
            )
            nc.vector.tensor_reduce(
                glob[:, 1:2], globs[:, j, :, 1], axis=AxisX, op=Alu.add
            )
            me = wpool.tile([P, 2], f32, name=f"me{j}")  # [mean, ex2]
            nc.vector.tensor_scalar_mul(me[:], glob[:], 1.0 / COUNT)
            mean = me[:, 0:1]
            # tmp = mean^2 - eps;  vpe = ex2 - tmp = var + eps;  rvpe = 1/vpe
            tmp = wpool.tile([P, 1], f32, name=f"tmp{j}")
            nc.vector.tensor_scalar(tmp[:], mean, mean, neg_eps[:], Alu.mult, Alu.add)
            vpe = wpool.tile([P, 1], f32, name=f"vpe{j}")
            nc.vector.tensor_sub(vpe[:], me[:, 1:2], tmp[:])
            rvpe = wpool.tile([P, 1], f32, name=f"rvpe{j}")
            nc.vector.reciprocal(rvpe[:], vpe[:])
            return mean, rvpe

        def norm_mid(j, rvpe):
            rsd = wpool.tile([P, 1], f32, name=f"rsd{j}")
            nc.scalar.activation(rsd[:], rvpe[:], Act.Sqrt)
            return rsd

        def norm_post(j, mean, rsd):
            scl = wpool.tile([P, 1], f32, name=f"scl{j}")
            nc.vector.tensor_mul(scl[:], rsd[:], gamma_sb[:, j : j + 1])
            mscl = wpool.tile([P, 1], f32, name=f"mscl{j}")
            nc.vector.tensor_mul(mscl[:], mean, scl[:])
            bia = wpool.tile([P, 1], f32, name=f"bia{j}")
            nc.vector.tensor_sub(bia[:], beta_sb[:, j : j + 1], mscl[:])
            return scl, bia

        HH = HW // 2

        def apply_slice(j, b, h, eng, scl, bia):
            # y = relu(scale*out + bias) -> bf16 y_sb (stores are emitted
            # separately on the sync/gpsimd rings). gpsimd elementwise is
            # ~10x slower than ACT/DVE and starves DVE's SBUF port - never
            # put apply work there.
            src = out_sb[:, b, j, h * HH : (h + 1) * HH]
            dst = y_sb[:, b, j, h * HH : (h + 1) * HH]
            if eng == "s":
                nc.scalar.activation(dst, src, Act.Relu, bias=bia[:], scale=scl[:])
            else:
                nc.vector.tensor_scalar(dst, src, scl[:], bia[:], Alu.mult, Alu.add)
                nc.vector.tensor_scalar_max(dst, dst, 0.0)

        def apply_chunk(j, scl, bia):
            # 8 half-image slices fanned ACT(4, fused relu) / DVE(4)
            plan = ["s", "v", "s", "v", "s", "v", "s", "v"]
            for i, eng in enumerate(plan):
                apply_slice(j, i // 2, i % 2, eng, scl, bia)

        def store_chunk(j, engs):
            # 8 half-image bf16 stores round-robined over otherwise-idle rings
            for i in range(8):
                b, h = i // 2, i % 2
                engs[i % len(engs)].dma_start(
                    y_flat[b, j * P : (j + 1) * P, h * HH : (h + 1) * HH],
                    y_sb[:, b, j, h * HH : (h + 1) * HH],
                )

        # ------------------------------------------------------------------
        # Emission. Every engine queue is strict in-order; the interleaving
        # below is the schedule (see module docstring for the timeline).
        # ------------------------------------------------------------------

        # gpsimd first: halo memsets (x_sign can't start until its xpad's
        # pad cells are zeroed; SWDGE gamma/beta issues must queue AFTER)
        for b in range(B_LOC):
            halo_memset(b)
        nc.gpsimd.dma_start(gamma_sb[:], g_d.ap().rearrange("(j p) -> p j", p=P))
        nc.gpsimd.dma_start(beta_sb[:], be_d.ap().rearrange("(j p) -> p j", p=P))
        # dummy warmup collective: absorbs the ~11us ncfw wakeup so the
        # real AllGathers start within ~1us of their triggers
        warm_in = dram.tile([P, 1], f32, name="warm_in")
        warm_out = dram.tile(
            [N_CORES * P, 1], f32, name="warm_out", addr_space="Shared"
        )
        nc.gpsimd.dma_start(warm_in[:], eps_sb[:])
        nc.gpsimd.collective_compute(
            "AllGather",
            Alu.bypass,
            replica_groups=[list(range(N_CORES))],
            ins=[warm_in.opt()],
            outs=[warm_out.opt()],
        )

        # sync ring: w0, w1 first (w1 -> alpha by ~13us; evictions need it at
        # ~20us), then img0 in quarter slices, imgs 1-3 in halves.
        w_dma(0)
        w_dma(1)
        Q = H // 4
        for q in range(4):
            x_load_rows(0, q * Q, (q + 1) * Q)
        for b in range(1, B_LOC):
            x_load_rows(b, 0, H // 2)
            x_load_rows(b, H // 2, H)

        # ScalarE: chunk-0 w signs back-to-back (transposes+fp8 casts
        # pipeline behind them on TE/DVE), then img0 signs at quarter grain.
        sgn0 = sgt.tile([P, K * K, C], bf16, tag="sgn", name="sgn0")
        pts0 = []
        for t in range(K * K):
            w_sign_tap(0, sgn0, t)
            pts0 += w_transpose_tap(0, sgn0, t)
        for q in range(4):
            x_sign_rows(0, q * Q, (q + 1) * Q)

        # DVE: the fp8 casts gate the first matmuls - they go FIRST; the clip
        # chains only feed alpha, needed at the first eviction (~10us later).
        # (clip TS ops wait on the w DMAs, which finish late because w and x
        # loads share HBM bandwidth - queueing them before the casts stalled
        # TensorE for 10.7us.)
        w_transpose_copy(0, pts0)
        w_clip_reduce(0)
        w_clip_reduce(1)
        alpha_finalize()
        # preload the Sqrt ACT table while ScalarE has slack so the BN norm
        # chain doesn't eat a 1.3us ACT_TABLE_LOAD on the post-collective
        # critical path
        sqrt_warm = wpool.tile([P, 1], f32, name="sqrt_warm")
        nc.scalar.activation(sqrt_warm[:], eps_sb[:], Act.Sqrt)

        # Unit order: all chunk-0 units first so AllGather(0) launches
        # mid-kernel; ALL chunk-1 conv/stats work is emitted before any
        # AllGather(0)-gated op so a late collective can never stall the
        # conv-critical ACT/DVE queues. Chunk-0 normalize+apply runs inside
        # the AllGather(1) wait; only its stores + chunk-1's tail are exposed.
        mm00 = conv_matmuls(0, 0)
        x_sign_rows(1, 0, H // 2)
        x_sign_rows(1, H // 2, H)
        conv_evict(0, 0, mm00)
        mm01 = conv_matmuls(0, 1)
        x_sign_rows(2, 0, H // 2)
        x_sign_rows(2, H // 2, H)
        conv_evict(0, 1, mm01)
        mm02 = conv_matmuls(0, 2)
        sq_big(0, 0, 0)
        x_sign_rows(3, 0, H // 2)
        x_sign_rows(3, H // 2, H)
        conv_evict(0, 2, mm02)
        sq_big(0, 1, 1)
        sgn1 = sgt.tile([P, K * K, C], bf16, tag="sgn", name="sgn1")
        for t in range(K * K):
            w_sign_tap(1, sgn1, t)
        mm03 = conv_matmuls(0, 3)
        conv_evict(0, 3, mm03)
        sq_big(0, 2, 2)
        stat_prereduce(0)
        sq_tiles(0, 3)
        stat_reduce(0)
        cc0 = launch_cc(0)

        pts1 = []
        for t in range(K * K):
            pts1 += w_transpose_tap(1, sgn1, t)
        w_transpose_copy(1, pts1)
        mm10 = conv_matmuls(1, 0)
        conv_evict(1, 0, mm10)
        sq_big(1, 0, 0)
        glob_ret(0, cc0)  # slot DMAs on sync+gpsimd, idle by now
        mm11 = conv_matmuls(1, 1)
        conv_evict(1, 1, mm11)
        sq_big(1, 1, 1)
        mm12 = conv_matmuls(1, 2)
        conv_evict(1, 2, mm12)
        sq_big(1, 2, 2)
        stat_prereduce(1)
        mm13 = conv_matmuls(1, 3)
        conv_evict(1, 3, mm13)
        sq_tiles(1, 3)
        stat_reduce(1)
        cc1 = launch_cc(1)

        # chunk-0 normalize+apply: emitted after the AllGather(1) launch so
        # every conv-critical op is already queued ahead of it; it executes
        # inside the AllGather(1) wait window.
        mean0, var0 = norm_pre(0)
        sd0 = norm_mid(0, var0)
        scl0, bia0 = norm_post(0, mean0, sd0)
        apply_chunk(0, scl0, bia0)
        store_chunk(0, [nc.sync])

        glob_ret(1, cc1)
        mean1, var1 = norm_pre(1)
        sd1 = norm_mid(1, var1)
        scl1, bia1 = norm_post(1, mean1, sd1)
        apply_chunk(1, scl1, bia1)
        # tail stores on sync+gpsimd so apply1's ACT slices never interleave
        # with store-issue instructions on the scalar queue
        store_chunk(1, [nc.sync, nc.gpsimd])

    nc.compile()
    return nc


def _get_nc():
    if "nc" not in _CACHE:
        _CACHE["nc"] = _build_nc()
    return _CACHE["nc"]


def _run(in_maps, trace=False, tmpdir=None):
    import concourse.bass_utils as bass_utils

    nc = _get_nc()
    return bass_utils.run_bass_kernel_spmd(
        nc, in_maps, core_ids=list(range(N_CORES)), trace=trace, tmpdir=tmpdir
    )


def _make_in_maps(x, w, gamma, beta):
    x = np.ascontiguousarray(np.asarray(x), dtype=np.float32)
    w = np.ascontiguousarray(np.asarray(w), dtype=np.float32)
    gamma = np.ascontiguousarray(np.asarray(gamma), dtype=np.float32)
    beta = np.ascontiguousarray(np.asarray(beta), dtype=np.float32)
    assert x.shape == (B, C, H, W)
    xs = np.split(x, N_CORES, axis=0)
    return [
        {"x": xs[i], "w": w, "gamma": gamma, "beta": beta} for i in range(N_CORES)
    ]


def _gather_y(res):
    ys = [np.asarray(r["y"]) for r in res.results]
    return np.concatenate(ys, axis=0).astype(np.float32)


def kernel(x, w, gamma, beta):
    in_maps = _make_in_maps(x, w, gamma, beta)
    res = _run(in_maps, trace=False)
    return _gather_y(res)


# ---- profiling helpers (used by test.py only) -------------------------

def _install_ntff_hook_shim():
    """bass_utils wants antenv.axon_hooks for NTFF tracing under axon; shim it."""
    import sys
    import types

    import antenv

    if "antenv.axon_hooks" in sys.modules:
        return
    mod = types.ModuleType("antenv.axon_hooks")
    mod._hook = None
    mod.set_axon_ntff_profile_hook = lambda h: setattr(mod, "_hook", h)
    mod.get_axon_ntff_profile_hook = lambda: mod._hook
    sys.modules["antenv.axon_hooks"] = mod
    antenv.axon_hooks = mod

    from trn_agent_boot.trn_boot import _ntff_profile_via_ctypes

    mod.set_axon_ntff_profile_hook(
        _ntff_profile_via_ctypes("/opt/axon/libaxon_pjrt.so")
    )


def kernel_traced(x, w, gamma, beta, tmpdir=None):
    """Run once with NTFF profiling; returns (y_full, exec_time_ns, trace_path)."""
    import concourse.bass_utils as bass_utils

    _install_ntff_hook_shim()
    bass_utils.upload_artifacts = lambda d: "local://disabled"
    in_maps = _make_in_maps(x, w, gamma, beta)
    res = _run(in_maps, trace=True, tmpdir=tmpdir)
    y = _gather_y(res)
    trace_path = (
        res.instructions_and_trace[1] if res.instructions_and_trace else None
    )
    return y, res.exec_time_ns, trace_path
